# revision 6
# baseline (speedup 1.0000x reference)
"""Trainium2 Bass kernel for nn_Contrast_54631984005844.

Strategy (8 NeuronCores, SPMD, two launches):

Launch 1 (projection, row-sharded): core k owns rows R_k = [512k, 512k+512).
  Computes the 2-layer MLP projection for the four z tensors on its row
  block, features on partitions (contraction chains without transposes).
  Stage 1 matmuls run fp8e4 DoubleRow (2 k-chunks per call); stage 2 runs
  bf16. ELU is exp(min(x,0)) + relu(x) = min(exp(x),1) + relu(x), so the
  stored h1 is elu(x)+1 and the layer-2 bias is compensated (b2a).
  The z_sc pair is fully normalized on chip (projT = 8*y/|y| in fp8); the
  z_mp pair is dumped raw in fp8 plus a 1/|y| vector (folded into L2's
  per-partition exp scale), which skips half the normalize multiplies.

Launch 2 (main, row-sharded): core k computes its 512x4096 row block of the
  TWO similarity matrices with fp8 DoubleRow matmuls, applies
  exp(s / (8*tau*|ym_i|)) on ACT with a per-partition scale AP (row sums
  fused via accum_out), and DUMPS the m blocks to DRAM in fp8. The host
  does the sparse pos-mask reductions (num/numt) and column sums from the
  dumped m - no pos-mask DMA, no DVE mask work, no gpsimd reductions.
  C = A @ B.T (A/B = host-computed raw sums, fp8) is also fp8 DoubleRow;
  the host gathers the 2x67000 indexed elements and applies log-sigmoid.
"""

import numpy as np
import ml_dtypes

import concourse.mybir as mybir
import concourse.tile as tile
from concourse import bacc
from concourse import bass_isa
from concourse.bass_utils import run_bass_kernel_spmd

BF16 = mybir.dt.bfloat16
F8 = mybir.dt.float8e4
F32 = mybir.dt.float32
AF = mybir.ActivationFunctionType
ALU = mybir.AluOpType
DR = mybir.MatmulPerfMode.DoubleRow

NPBF16 = ml_dtypes.bfloat16
NPF8 = ml_dtypes.float8_e4m3

N = 4096          # rows per view
H = 512           # hidden dim
NC = 8            # cores
RB = N // NC      # row block = 512
TAU = 0.8
LAM = 0.5
KC = H // 128     # 4 contraction chunks
AT = RB // 128    # 4 a-tiles
HW = 2048         # half width for the 4096-wide sweeps
NH = N // HW      # 2 halves
PW = 2 * RB       # 1024: two tensors batched along free dim (one pair)

_CACHE = {}


# --------------------------------------------------------------------------
# Launch 1: projection
# --------------------------------------------------------------------------

def _build_l1():
    nc = bacc.Bacc(None, target_bir_lowering=False, debug=False)

    # zt order: [z_sc1, z_sc2, z_mp1, z_mp2] (zs pair first, zm pair second)
    zt_d = nc.declare_dram_parameter("zt", [4, H, RB], F8, isOutput=False)
    w1t_d = nc.declare_dram_parameter("w1t", [H, H], F8, isOutput=False)
    w2t_d = nc.declare_dram_parameter("w2t", [H, H], BF16, isOutput=False)
    b1_d = nc.declare_dram_parameter("b1", [H, 1], F32, isOutput=False)
    b2a_d = nc.declare_dram_parameter("b2a", [H, 1], F32, isOutput=False)

    # projT: [0]=8*zs1n, [1]=8*zs2n (fp8), [2]=ym1 raw, [3]=ym2 raw (fp8)
    projT_d = nc.declare_dram_parameter("projT", [4, H, RB], F8, isOutput=True)
    # nminv: [1, 1024] f32 = 8/|ym| for [ym1(512) | ym2(512)]
    nminv_d = nc.declare_dram_parameter("nminv", [1, PW], F32, isOutput=True)

    with tile.TileContext(nc) as tc:
        with (
            tc.tile_pool(name="const", bufs=1) as cpool,
            tc.tile_pool(name="sb", bufs=2) as sb,
            tc.tile_pool(name="work", bufs=3) as work,
            tc.tile_pool(name="nrm", bufs=2) as nrm,
            tc.tile_pool(name="ps", bufs=2, space="PSUM") as ps,
        ):
            w1sb = cpool.tile([128, KC, H], F8, tag="w1", name="w1sb")
            w2sb = cpool.tile([128, KC, H], BF16, tag="w2", name="w2sb")
            b1sb = cpool.tile([128, KC], F32, tag="b1", name="b1sb")
            b2sb = cpool.tile([128, KC], F32, tag="b2", name="b2sb")
            ztall = cpool.tile([128, KC, 2 * PW], F8, tag="ztall", name="ztall")
            nc.sync.dma_start(w1sb[:], w1t_d[:].rearrange("(a p) o -> p a o", p=128))
            nc.sync.dma_start(b1sb[:], b1_d[:].rearrange("(a p) one -> p (a one)", p=128))
            nc.sync.dma_start(b2sb[:], b2a_d[:].rearrange("(a p) one -> p (a one)", p=128))
            for t in range(4):
                nc.sync.dma_start(ztall[:, :, t * RB:(t + 1) * RB],
                                  zt_d[t].rearrange("(a p) r -> p a r", p=128))
            nc.sync.dma_start(w2sb[:], w2t_d[:].rearrange("(a p) o -> p a o", p=128))

            for pr in range(2):           # 0 = zs pair (normalized), 1 = zm pair
                o = pr * PW
                is_zs = pr == 0
                # ---- stage 1: h1 = elu(z @ W1.T + b1) + 1 ----
                h1 = sb.tile([128, KC, PW], BF16, tag="h1", name=f"h1_{pr}")
                for oc in range(KC):
                    p1 = ps.tile([128, PW], F32, tag="p1", name=f"p1_{pr}_{oc}")
                    for tt in range(2):
                        for kp in range(2):
                            nc.tensor.matmul(
                                p1[:, tt * RB:(tt + 1) * RB],
                                w1sb[:, 2 * kp:2 * kp + 2, oc * 128:(oc + 1) * 128],
                                ztall[:, 2 * kp:2 * kp + 2,
                                      o + tt * RB:o + (tt + 1) * RB],
                                start=(kp == 0), stop=(kp == 1), perf_mode=DR)
                    bias = b1sb[:, oc:oc + 1]
                    e = work.tile([128, PW], BF16, tag="e", name=f"e_{pr}_{oc}")
                    nc.scalar.activation(e[:], p1[:], AF.Exp, bias=bias)
                    rl = work.tile([128, PW], BF16, tag="rl", name=f"rl_{pr}_{oc}")
                    if is_zs:
                        nc.scalar.activation(rl[:], p1[:], AF.Relu, bias=bias)
                    else:
                        nc.vector.tensor_scalar(rl[:], p1[:], bias, 0.0,
                                                ALU.add, ALU.max)
                    nc.vector.scalar_tensor_tensor(
                        h1[:, oc, :], e[:], 1.0, rl[:], ALU.min, ALU.add)

                # ---- stage 2: y = h1 @ W2.T + b2a ; squares for norms ----
                yb = sb.tile([128, KC, PW], BF16 if is_zs else F8,
                             tag="yb", name=f"yb_{pr}")
                sq = sb.tile([128, KC, PW], BF16, tag="sq", name=f"sq_{pr}")
                for oc in range(KC):
                    p2 = ps.tile([128, PW], F32, tag="p2", name=f"p2_{pr}_{oc}")
                    for tt in range(2):
                        for k in range(KC):
                            nc.tensor.matmul(
                                p2[:, tt * RB:(tt + 1) * RB],
                                w2sb[:, k, oc * 128:(oc + 1) * 128],
                                h1[:, k, tt * RB:(tt + 1) * RB],
                                start=(k == 0), stop=(k == KC - 1))
                    bias = b2sb[:, oc:oc + 1]
                    nc.scalar.activation(sq[:, oc, :], p2[:], AF.Square, bias=bias)
                    if is_zs:
                        nc.vector.tensor_scalar(yb[:, oc, :], p2[:], bias, None,
                                                ALU.add)
                    else:
                        # raw ym in fp8: this IS the projT payload
                        nc.scalar.activation(yb[:, oc, :], p2[:], AF.Identity,
                                             bias=bias)

                # ---- norms: |y|^2 summed over features (partitions x KC) ----
                sqs = nrm.tile([128, PW], BF16, tag="sqs", name=f"sqs_{pr}")
                sqa = nrm.tile([128, PW], BF16, tag="sqa", name=f"sqa_{pr}")
                nc.vector.tensor_tensor(sqa[:], sq[:, 0, :], sq[:, 1, :], ALU.add)
                nc.vector.tensor_tensor(sqs[:], sq[:, 2, :], sq[:, 3, :], ALU.add)
                nc.vector.tensor_tensor(sqs[:], sqs[:], sqa[:], ALU.add)
                nrmb = nrm.tile([128, PW], F32, tag="nrmb", name=f"nrmb_{pr}")
                nc.gpsimd.partition_all_reduce(nrmb[:], sqs[:], 128,
                                               bass_isa.ReduceOp.add)
                if is_zs:
                    snb = nrm.tile([128, PW], F32, tag="snb", name="snb")
                    nc.scalar.activation(snb[:], nrmb[:], AF.Sqrt, scale=1.0 / 64.0)
                    rnb = nrm.tile([128, PW], F32, tag="rnb", name="rnb")
                    nc.vector.reciprocal_approx_fast(rnb[:], snb[:])
                    zn = sb.tile([128, KC, PW], F8, tag="zn", name="zn")
                    for oc in range(KC):
                        nc.vector.tensor_tensor(zn[:, oc, :], yb[:, oc, :],
                                                rnb[:], ALU.mult)
                    for tt in range(2):
                        nc.sync.dma_start(
                            projT_d[tt].rearrange("(a p) r -> p a r", p=128),
                            zn[:, :, tt * RB:(tt + 1) * RB])
                else:
                    snb2 = nrm.tile([1, PW], F32, tag="snb2", name="snb2")
                    nc.scalar.activation(snb2[:], nrmb[0:1, :], AF.Sqrt,
                                         scale=1.0 / 64.0)
                    rnb2 = nrm.tile([1, PW], F32, tag="rnb2", name="rnb2")
                    nc.vector.reciprocal_approx_fast(rnb2[:], snb2[:])
                    nc.sync.dma_start(nminv_d[:], rnb2[:])
                    for tt in range(2):
                        nc.sync.dma_start(
                            projT_d[2 + tt].rearrange("(a p) r -> p a r", p=128),
                            yb[:, :, tt * RB:(tt + 1) * RB])

    nc.finalize()
    return nc


# --------------------------------------------------------------------------
# Launch 2: two similarity row-blocks (m dumped to DRAM) + C matrix
# --------------------------------------------------------------------------

def _build_l2():
    nc = bacc.Bacc(None, target_bir_lowering=False, debug=False)

    lm1_d = nc.declare_dram_parameter("lm1", [H, RB], F8, isOutput=False)
    lm2_d = nc.declare_dram_parameter("lm2", [H, RB], F8, isOutput=False)
    scl_d = nc.declare_dram_parameter("scl", [2, RB], F32, isOutput=False)
    r1_d = nc.declare_dram_parameter("r1", [H, N], F8, isOutput=False)
    r2_d = nc.declare_dram_parameter("r2", [H, N], F8, isOutput=False)
    atb_d = nc.declare_dram_parameter("atb", [H, RB], F8, isOutput=False)
    btf_d = nc.declare_dram_parameter("btf", [H, N], F8, isOutput=False)

    m1_d = nc.declare_dram_parameter("m1", [RB, N], F8, isOutput=True)
    m2_d = nc.declare_dram_parameter("m2", [RB, N], F8, isOutput=True)
    c_d = nc.declare_dram_parameter("c", [RB, N], F8, isOutput=True)
    rp_d = nc.declare_dram_parameter("rparts", [2, 128, 8], F32, isOutput=True)

    with tile.TileContext(nc) as tc:
        with (
            tc.tile_pool(name="res", bufs=1) as res,
            tc.tile_pool(name="rfull", bufs=1) as rfp,
            tc.tile_pool(name="acc", bufs=1) as accp,
            tc.tile_pool(name="mh", bufs=4) as mhp,
            tc.tile_pool(name="ps", bufs=2, space="PSUM") as ps,
        ):
            lm1 = res.tile([128, KC, RB], F8, tag="lm1", name="lm1")
            lm2 = res.tile([128, KC, RB], F8, tag="lm2", name="lm2")
            atb = res.tile([128, KC, RB], F8, tag="atb", name="atb")
            sclsb = res.tile([128, 2, AT], F32, tag="scl", name="sclsb")
            r1 = rfp.tile([128, KC, N], F8, tag="r1", name="r1")
            r2 = rfp.tile([128, KC, N], F8, tag="r2", name="r2")
            btf = rfp.tile([128, KC, N], F8, tag="btf", name="btf")
            nc.sync.dma_start(lm1[:], lm1_d[:].rearrange("(a p) r -> p a r", p=128))
            for s in range(2):
                nc.sync.dma_start(sclsb[:, s, :],
                                  scl_d[s].rearrange("(a p) -> p a", p=128))
            nc.sync.dma_start(r1[:], r1_d[:].rearrange("(a p) b -> p a b", p=128))
            nc.sync.dma_start(lm2[:], lm2_d[:].rearrange("(a p) r -> p a r", p=128))
            nc.sync.dma_start(r2[:], r2_d[:].rearrange("(a p) b -> p a b", p=128))
            nc.sync.dma_start(atb[:], atb_d[:].rearrange("(a p) r -> p a r", p=128))
            nc.sync.dma_start(btf[:], btf_d[:].rearrange("(a p) b -> p a b", p=128))

            rparts = accp.tile([128, 2, 8], F32, tag="rparts", name="rparts")

            for s, (lm, rr, m_d) in enumerate(
                ((lm1, r1, m1_d), (lm2, r2, m2_d))
            ):
                for a in range(AT):
                    for hf in range(NH):
                        slot = a * 2 + hf
                        pss = ps.tile([128, HW], F32, tag="pss", name="pss")
                        for n in range(HW // 512):
                            off = hf * HW + n * 512
                            for kp in range(2):
                                nc.tensor.matmul(
                                    pss[:, n * 512:(n + 1) * 512],
                                    lm[:, 2 * kp:2 * kp + 2,
                                       a * 128:(a + 1) * 128],
                                    rr[:, 2 * kp:2 * kp + 2, off:off + 512],
                                    start=(kp == 0), stop=(kp == 1),
                                    perf_mode=DR)
                        mh = mhp.tile([128, HW], F8, tag="mh", name="mh")
                        nc.scalar.activation(
                            mh[:], pss[:], AF.Exp,
                            scale=sclsb[:, s, a:a + 1],
                            accum_out=rparts[:, s, slot:slot + 1])
                        nc.sync.dma_start(
                            m_d[a * 128:(a + 1) * 128, hf * HW:(hf + 1) * HW],
                            mh[:])

            nc.sync.dma_start(rp_d[:].rearrange("s p e -> p s e"), rparts[:])

            # C = A @ B.T row block
            c_ap = c_d[:].rearrange("(a p) b -> p a b", p=128)
            for a in range(AT):
                for hf in range(NH):
                    psc = ps.tile([128, HW], F32, tag="pss", name="psc")
                    for n in range(HW // 512):
                        off = hf * HW + n * 512
                        for kp in range(2):
                            nc.tensor.matmul(
                                psc[:, n * 512:(n + 1) * 512],
                                atb[:, 2 * kp:2 * kp + 2, a * 128:(a + 1) * 128],
                                btf[:, 2 * kp:2 * kp + 2, off:off + 512],
                                start=(kp == 0), stop=(kp == 1), perf_mode=DR)
                    cb = mhp.tile([128, HW], F8, tag="cb", name="cb")
                    nc.vector.tensor_copy(cb[:], psc[:])
                    nc.sync.dma_start(c_ap[:, a, hf * HW:(hf + 1) * HW], cb[:])

    nc.finalize()
    return nc


# --------------------------------------------------------------------------
# Host orchestration
# --------------------------------------------------------------------------

def _get_programs():
    if "l1" not in _CACHE:
        _CACHE["l1"] = _build_l1()
    if "l2" not in _CACHE:
        _CACHE["l2"] = _build_l2()
    return _CACHE["l1"], _CACHE["l2"]


def _f8(x):
    return np.ascontiguousarray(np.asarray(x).astype(NPF8))


def _make_l1_inputs(z_mp1, z_sc1, z_mp2, z_sc2, W1, b1, W2, b2):
    # zt order: [z_sc1, z_sc2, z_mp1, z_mp2]
    zts = [_f8(z.T) for z in (z_sc1, z_sc2, z_mp1, z_mp2)]
    w1t = _f8(W1.T)
    w2t = np.ascontiguousarray(W2.T.astype(NPBF16))
    b1c = np.ascontiguousarray(b1.reshape(H, 1), dtype=np.float32)
    b2a = np.ascontiguousarray((b2 - W2.sum(axis=1)).reshape(H, 1),
                               dtype=np.float32)
    in1 = []
    for k in range(NC):
        sl = slice(k * RB, (k + 1) * RB)
        zt = np.ascontiguousarray(np.stack([z[:, sl] for z in zts]))
        in1.append({"zt": zt, "w1t": w1t, "w2t": w2t, "b1": b1c, "b2a": b2a})
    return in1


def _make_l2_inputs(res1, z_mp1, z_sc1, z_mp2, z_sc2):
    projT = [res1[k]["projT"] for k in range(NC)]
    r1f = np.ascontiguousarray(np.concatenate([p[0] for p in projT], axis=1))
    r2f = np.ascontiguousarray(np.concatenate([p[1] for p in projT], axis=1))

    A = z_mp1 + z_sc1
    B = z_mp2 + z_sc2
    atbf = _f8(A.T)
    btf = _f8(B.T)

    in2 = []
    for k in range(NC):
        sl = slice(k * RB, (k + 1) * RB)
        nminv = res1[k]["nminv"].reshape(2, RB).astype(np.float32)
        scl = np.ascontiguousarray(nminv / (64.0 * TAU))
        in2.append({
            "lm1": np.ascontiguousarray(projT[k][2]),
            "lm2": np.ascontiguousarray(projT[k][3]),
            "scl": scl,
            "r1": r1f, "r2": r2f,
            "atb": np.ascontiguousarray(atbf[:, sl]),
            "btf": btf,
        })
    return in2


def _finish(res2, pos1, pos2, pos_i, pos_j, neg_i, neg_j):
    def _vec(parts):  # [128, 8] slot = a*2+half -> [512]
        return parts.reshape(128, 4, 2).sum(axis=2).T.reshape(RB)

    losses = []
    for s, pos in ((0, pos1), (1, pos2)):
        key = "m1" if s == 0 else "m2"
        mf = np.concatenate(
            [res2[k][key].astype(np.float32) for k in range(NC)], axis=0)
        rsum = np.concatenate(
            [_vec(res2[k]["rparts"][s]) for k in range(NC)]).astype(np.float64)
        num = np.einsum("ij,ij->i", mf, pos, dtype=np.float64)
        csum = mf.sum(axis=0, dtype=np.float64)
        numt = np.einsum("ij,ji->j", mf, pos, dtype=np.float64)
        lori_mp = -np.log(num / (rsum + 1e-8)).mean()
        lori_sc = -np.log(numt / (csum + 1e-8)).mean()
        losses.append(LAM * lori_mp + (1.0 - LAM) * lori_sc)

    C = np.concatenate([res2[k]["c"].astype(np.float32) for k in range(NC)],
                       axis=0)
    ip1 = C[pos_i, pos_j].astype(np.float64)
    ip2 = C[neg_i, neg_j].astype(np.float64)

    def logsig(x):
        return -np.logaddexp(0.0, -x)

    loss_main = -logsig(ip1).mean() + logsig(-ip2).mean()
    return np.float32(loss_main + losses[0] + losses[1])


def kernel(z_mp1, z_sc1, pos1, z_mp2, z_sc2, pos2,
           W1, b1, W2, b2, pos_i, pos_j, neg_i, neg_j):
    z_mp1 = np.asarray(z_mp1, np.float32)
    z_sc1 = np.asarray(z_sc1, np.float32)
    z_mp2 = np.asarray(z_mp2, np.float32)
    z_sc2 = np.asarray(z_sc2, np.float32)
    pos1 = np.asarray(pos1, np.float32)
    pos2 = np.asarray(pos2, np.float32)
    W1 = np.asarray(W1, np.float32)
    W2 = np.asarray(W2, np.float32)
    b1 = np.asarray(b1, np.float32)
    b2 = np.asarray(b2, np.float32)
    pos_i = np.asarray(pos_i)
    pos_j = np.asarray(pos_j)
    neg_i = np.asarray(neg_i)
    neg_j = np.asarray(neg_j)

    l1, l2 = _get_programs()
    cores = list(range(NC))

    in1 = _make_l1_inputs(z_mp1, z_sc1, z_mp2, z_sc2, W1, b1, W2, b2)
    res1 = run_bass_kernel_spmd(l1, in1, cores).results

    in2 = _make_l2_inputs(res1, z_mp1, z_sc1, z_mp2, z_sc2)
    res2 = run_bass_kernel_spmd(l2, in2, cores).results

    return _finish(res2, pos1, pos2, pos_i, pos_j, neg_i, neg_j)


# revision 8
# speedup vs baseline: 1.0999x; 1.0999x over previous
"""Trainium2 Bass kernel for nn_Contrast_54631984005844.

Strategy (8 NeuronCores, SPMD, two launches):

Launch 1 (projection, row-sharded): core k owns rows R_k = [512k, 512k+512).
  Stage 1 (feature-major): h1 = elu(z @ W1.T + b1) + 1 with fp8 DoubleRow
  matmuls; ELU as min(exp(x),1) + relu(x) (elu+1, layer-2 bias compensated).
  Stage 2 (sample-major): y.T-block @ W2.T with h1 as the stationary operand
  so the psum comes out [samples, features]; the layer-2 bias is folded in
  as an extra 1-partition bf16 matmul row. Norms are then free-dim reductions
  (accum_out), and the normalize multiply is a per-partition scale - no wide
  norm pipeline. The z_sc pair is normalized on chip (8*y/|y| in fp8); the
  z_mp pair is dumped raw in fp8 + its |y|^2 vector (folded into L2's
  per-partition exp scale on the host).

Launch 2 (main, row-sharded): core k computes its 512x4096 row block of the
  TWO similarity matrices with fp8 DoubleRow matmuls, applies
  exp(s / (8*tau*|ym_i|)) on ACT with a per-partition scale AP, and DUMPS
  the m blocks to DRAM in fp8. The host does all pos-mask reductions
  (num/numt) plus row/column sums from the dumped m - no pos-mask DMA, no
  on-chip mask work. C = A @ B.T (A/B host-computed raw sums, fp8) runs
  interleaved with the view-2 similarity tiles (PE+DVE) under the ACT-paced
  exp stream. All DMAs are host-pre-permuted to 128-descriptor contiguous
  transfers.
"""

import numpy as np
import ml_dtypes

import concourse.mybir as mybir
import concourse.tile as tile
from concourse import bacc
from concourse.bass_utils import run_bass_kernel_spmd

BF16 = mybir.dt.bfloat16
F8 = mybir.dt.float8e4
F32 = mybir.dt.float32
AF = mybir.ActivationFunctionType
ALU = mybir.AluOpType
DR = mybir.MatmulPerfMode.DoubleRow

NPBF16 = ml_dtypes.bfloat16
NPF8 = ml_dtypes.float8_e4m3

N = 4096          # rows per view
H = 512           # hidden dim
NC = 8            # cores
RB = N // NC      # row block = 512
TAU = 0.8
LAM = 0.5
KC = H // 128     # 4 contraction chunks
AT = RB // 128    # 4 sample-tiles per tensor
HW = 2048         # half width for the 4096-wide sweeps
NH = N // HW      # 2 halves
PW = 2 * RB       # 1024: two tensors batched along free dim (one pair)

_CACHE = {}


# --------------------------------------------------------------------------
# Launch 1: projection
# --------------------------------------------------------------------------

def _build_l1():
    nc = bacc.Bacc(None, target_bir_lowering=False, debug=False)

    # zt order: [z_sc1, z_sc2, z_mp1, z_mp2]; layout [128, t, kc, r] p-major
    zt_d = nc.declare_dram_parameter("zt", [128, 4 * KC * RB], F8, isOutput=False)
    w1t_d = nc.declare_dram_parameter("w1t", [128, KC * H], F8, isOutput=False)
    w2t_d = nc.declare_dram_parameter("w2t", [128, KC * H], BF16, isOutput=False)
    b1_d = nc.declare_dram_parameter("b1", [128, KC], F32, isOutput=False)
    b2r_d = nc.declare_dram_parameter("b2r", [1, H], BF16, isOutput=False)
    ones_d = nc.declare_dram_parameter("ones", [1, 128], BF16, isOutput=False)

    # pj[t]: sample-major y-block: pj[t][p, st*H+f] = row (st*128+p), feat f
    # t=0,1: 8*y/|y| for z_sc1/z_sc2 ; t=2,3: raw y for z_mp1/z_mp2
    pj_d = nc.declare_dram_parameter("pj", [4, 128, AT * H], F8, isOutput=True)
    # nrm cols for the zm pair: |y|^2 per sample, col = tl*AT+st
    nrm_d = nc.declare_dram_parameter("nrm", [128, 2 * AT], F32, isOutput=True)

    with tile.TileContext(nc) as tc:
        with (
            tc.tile_pool(name="const", bufs=1) as cpool,
            tc.tile_pool(name="sb", bufs=2) as sb,
            tc.tile_pool(name="work", bufs=3) as work,
            tc.tile_pool(name="scr", bufs=2) as scrp,
            tc.tile_pool(name="nrm", bufs=1) as nrmp,
            tc.tile_pool(name="ps1", bufs=1, space="PSUM") as ps1,
            tc.tile_pool(name="ps2", bufs=6, space="PSUM") as ps2,
        ):
            w1sb = cpool.tile([128, KC, H], F8, tag="w1", name="w1sb")
            b1sb = cpool.tile([128, KC], F32, tag="b1", name="b1sb")
            ztall = cpool.tile([128, 2, 2, KC, RB], F8, tag="ztall", name="ztall")
            w2sb = cpool.tile([128, KC, H], BF16, tag="w2", name="w2sb")
            b2row = cpool.tile([1, H], BF16, tag="b2r", name="b2row")
            onesb = cpool.tile([1, 128], BF16, tag="ones", name="onesb")
            nc.sync.dma_start(w1sb[:], w1t_d[:].rearrange("p (a o) -> p a o", a=KC))
            nc.sync.dma_start(b1sb[:], b1_d[:])
            nc.sync.dma_start(
                ztall[:, 0], zt_d[:, :2 * KC * RB].rearrange(
                    "p (t a r) -> p t a r", t=2, a=KC))
            nc.sync.dma_start(w2sb[:], w2t_d[:].rearrange("p (a o) -> p a o", a=KC))
            nc.sync.dma_start(b2row[:], b2r_d[:])
            nc.sync.dma_start(onesb[:], ones_d[:])
            nc.sync.dma_start(
                ztall[:, 1], zt_d[:, 2 * KC * RB:].rearrange(
                    "p (t a r) -> p t a r", t=2, a=KC))

            for pr in range(2):           # 0 = zs pair (normalized), 1 = zm pair
                is_zs = pr == 0
                # ---- stage 1 (feature-major): h1 = elu(z @ W1.T + b1) + 1 ----
                h1 = sb.tile([128, KC, PW], BF16, tag="h1", name=f"h1_{pr}")
                for oc in range(KC):
                    p1 = ps1.tile([128, PW], F32, tag="p1", name=f"p1_{pr}_{oc}")
                    for tt in range(2):
                        for kp in range(2):
                            nc.tensor.matmul(
                                p1[:, tt * RB:(tt + 1) * RB],
                                w1sb[:, 2 * kp:2 * kp + 2, oc * 128:(oc + 1) * 128],
                                ztall[:, pr, tt, 2 * kp:2 * kp + 2, :],
                                start=(kp == 0), stop=(kp == 1), perf_mode=DR)
                    bias = b1sb[:, oc:oc + 1]
                    e = work.tile([128, PW], BF16, tag="e", name=f"e_{pr}_{oc}")
                    nc.scalar.activation(e[:], p1[:], AF.Exp, bias=bias)
                    rl = work.tile([128, PW], BF16, tag="rl", name=f"rl_{pr}_{oc}")
                    if is_zs:
                        nc.scalar.activation(rl[:], p1[:], AF.Relu, bias=bias)
                    else:
                        nc.vector.tensor_scalar(rl[:], p1[:], bias, 0.0,
                                                ALU.add, ALU.max)
                    nc.vector.scalar_tensor_tensor(
                        h1[:, oc, :], e[:], 1.0, rl[:], ALU.min, ALU.add)

                # ---- stage 2 (sample-major): y = h1.T-block @ W2.T + b2a ----
                nrmc = nrmp.tile([128, 2 * AT], F32, tag=f"nrmc{pr}", name=f"nrmc{pr}")
                snc = nrmp.tile([128, 2 * AT], F32, tag=f"snc{pr}", name=f"snc{pr}")
                rnbc = nrmp.tile([128, 2 * AT], F32, tag=f"rnbc{pr}", name=f"rnbc{pr}")
                zn = sb.tile([128, 2, AT, H], F8, tag="zn", name=f"zn_{pr}")
                p2s = {}
                for tl in range(2):
                    o = tl * RB
                    for st in range(AT):
                        idx = tl * AT + st
                        p2 = ps2.tile([128, H], F32, tag="p2", name=f"p2_{pr}_{idx}")
                        p2s[idx] = p2
                        for k in range(KC):
                            nc.tensor.matmul(
                                p2[:],
                                h1[:, k, o + st * 128:o + (st + 1) * 128],
                                w2sb[:, k, :],
                                start=(k == 0), stop=False)
                        nc.tensor.matmul(p2[:], onesb[:], b2row[:],
                                         start=False, stop=True)
                        scr = scrp.tile([128, H], BF16, tag="scr", name=f"s_{pr}_{idx}")
                        if idx % 2 == 0:
                            nc.scalar.activation(
                                scr[:], p2[:], AF.Square,
                                accum_out=nrmc[:, idx:idx + 1])
                        else:
                            nc.vector.tensor_tensor_reduce(
                                scr[:], p2[:], p2[:], 1.0, 0.0,
                                ALU.mult, ALU.add,
                                accum_out=nrmc[:, idx:idx + 1])
                    if is_zs:
                        sl = slice(tl * AT, (tl + 1) * AT)
                        nc.scalar.activation(snc[:, sl], nrmc[:, sl], AF.Sqrt,
                                             scale=1.0 / 64.0)
                        nc.vector.reciprocal_approx_fast(rnbc[:, sl], snc[:, sl])
                        for st in range(AT):
                            idx = tl * AT + st
                            p2 = p2s[idx]
                            if st % 2 == 0:
                                nc.scalar.activation(
                                    zn[:, tl, st, :], p2[:], AF.Identity,
                                    scale=rnbc[:, idx:idx + 1])
                            else:
                                nc.vector.tensor_scalar(
                                    zn[:, tl, st, :], p2[:],
                                    rnbc[:, idx:idx + 1], None, ALU.mult)
                    else:
                        for st in range(AT):
                            idx = tl * AT + st
                            p2 = p2s[idx]
                            if st % 2 == 0:
                                nc.scalar.activation(zn[:, tl, st, :], p2[:],
                                                     AF.Identity)
                            else:
                                nc.vector.tensor_copy(zn[:, tl, st, :], p2[:])
                    nc.sync.dma_start(
                        pj_d[2 * pr + tl].rearrange("p (a f) -> p a f", a=AT),
                        zn[:, tl])
                if not is_zs:
                    nc.sync.dma_start(nrm_d[:], nrmc[:])

    nc.finalize()
    return nc


# --------------------------------------------------------------------------
# Launch 2: two similarity row-blocks (m dumped to DRAM) + C matrix
# --------------------------------------------------------------------------

def _build_l2():
    nc = bacc.Bacc(None, target_bir_lowering=False, debug=False)

    lm1_d = nc.declare_dram_parameter("lm1", [128, KC * RB], F8, isOutput=False)
    lm2_d = nc.declare_dram_parameter("lm2", [128, KC * RB], F8, isOutput=False)
    scl_d = nc.declare_dram_parameter("scl", [128, 2 * AT], F32, isOutput=False)
    r1_d = nc.declare_dram_parameter("r1", [128, NH * KC * HW], F8, isOutput=False)
    r2_d = nc.declare_dram_parameter("r2", [128, NH * KC * HW], F8, isOutput=False)
    atb_d = nc.declare_dram_parameter("atb", [128, KC * RB], F8, isOutput=False)
    btf_d = nc.declare_dram_parameter("btf", [128, NH * KC * HW], F8, isOutput=False)

    # m/c dumps: slot-major [a*2+hf, 128, HW]
    m1_d = nc.declare_dram_parameter("m1", [2 * AT, 128, HW], F8, isOutput=True)
    m2_d = nc.declare_dram_parameter("m2", [2 * AT, 128, HW], F8, isOutput=True)
    c_d = nc.declare_dram_parameter("c", [2 * AT, 128, HW], F8, isOutput=True)

    with tile.TileContext(nc) as tc:
        with (
            tc.tile_pool(name="res", bufs=1) as res,
            tc.tile_pool(name="rfull", bufs=1) as rfp,
            tc.tile_pool(name="mh", bufs=4) as mhp,
            tc.tile_pool(name="ps", bufs=2, space="PSUM") as ps,
        ):
            lm1 = res.tile([128, KC, RB], F8, tag="lm1", name="lm1")
            lm2 = res.tile([128, KC, RB], F8, tag="lm2", name="lm2")
            atb = res.tile([128, KC, RB], F8, tag="atb", name="atb")
            sclsb = res.tile([128, 2, AT], F32, tag="scl", name="sclsb")
            r1 = [rfp.tile([128, KC, HW], F8, tag=f"r1_{h}", name=f"r1_{h}")
                  for h in range(NH)]
            r2 = [rfp.tile([128, KC, HW], F8, tag=f"r2_{h}", name=f"r2_{h}")
                  for h in range(NH)]
            btf = [rfp.tile([128, KC, HW], F8, tag=f"bt_{h}", name=f"bt_{h}")
                   for h in range(NH)]
            CH = KC * HW
            nc.sync.dma_start(lm1[:], lm1_d[:].rearrange("p (a r) -> p a r", a=KC))
            nc.sync.dma_start(sclsb[:], scl_d[:].rearrange("p (s a) -> p s a", s=2))
            for h in range(NH):
                nc.sync.dma_start(
                    r1[h][:], r1_d[:, h * CH:(h + 1) * CH].rearrange(
                        "p (a b) -> p a b", a=KC))
            nc.sync.dma_start(atb[:], atb_d[:].rearrange("p (a r) -> p a r", a=KC))
            for h in range(NH):
                nc.sync.dma_start(
                    btf[h][:], btf_d[:, h * CH:(h + 1) * CH].rearrange(
                        "p (a b) -> p a b", a=KC))
            nc.sync.dma_start(lm2[:], lm2_d[:].rearrange("p (a r) -> p a r", a=KC))
            for h in range(NH):
                nc.sync.dma_start(
                    r2[h][:], r2_d[:, h * CH:(h + 1) * CH].rearrange(
                        "p (a b) -> p a b", a=KC))

            def sim_tile(s, lm, rr, m_d, a, hf):
                slot = a * 2 + hf
                pss = ps.tile([128, HW], F32, tag="pss", name=f"ps{s}_{slot}")
                for n in range(HW // 512):
                    for kp in range(2):
                        nc.tensor.matmul(
                            pss[:, n * 512:(n + 1) * 512],
                            lm[:, 2 * kp:2 * kp + 2, a * 128:(a + 1) * 128],
                            rr[hf][:, 2 * kp:2 * kp + 2, n * 512:(n + 1) * 512],
                            start=(kp == 0), stop=(kp == 1), perf_mode=DR)
                mh = mhp.tile([128, HW], F8, tag="mh", name="mh")
                nc.scalar.activation(mh[:], pss[:], AF.Exp,
                                     scale=sclsb[:, s, a:a + 1])
                nc.sync.dma_start(m_d[slot], mh[:])

            def c_tile(a, hf):
                slot = a * 2 + hf
                psc = ps.tile([128, HW], F32, tag="pss", name=f"psc_{slot}")
                for n in range(HW // 512):
                    for kp in range(2):
                        nc.tensor.matmul(
                            psc[:, n * 512:(n + 1) * 512],
                            atb[:, 2 * kp:2 * kp + 2, a * 128:(a + 1) * 128],
                            btf[hf][:, 2 * kp:2 * kp + 2, n * 512:(n + 1) * 512],
                            start=(kp == 0), stop=(kp == 1), perf_mode=DR)
                cb = mhp.tile([128, HW], F8, tag="cb", name="cb")
                nc.vector.tensor_copy(cb[:], psc[:])
                nc.sync.dma_start(c_d[slot], cb[:])

            for a in range(AT):
                for hf in range(NH):
                    sim_tile(0, lm1, r1, m1_d, a, hf)
            for a in range(AT):
                for hf in range(NH):
                    c_tile(a, hf)
                    sim_tile(1, lm2, r2, m2_d, a, hf)

    nc.finalize()
    return nc


# --------------------------------------------------------------------------
# Host orchestration
# --------------------------------------------------------------------------

def _get_programs():
    if "l1" not in _CACHE:
        _CACHE["l1"] = _build_l1()
    if "l2" not in _CACHE:
        _CACHE["l2"] = _build_l2()
    return _CACHE["l1"], _CACHE["l2"]


def _f8(x):
    return np.ascontiguousarray(np.asarray(x).astype(NPF8))


def _fm_pmajor(M):
    """[rows, H] feature-major p-major: out[p, kc*rows + r] = M[r, kc*128+p]."""
    R = M.shape[0]
    return np.ascontiguousarray(
        M.reshape(R, KC, 128).transpose(2, 1, 0).reshape(128, KC * R))


def _r_pmajor(Z):
    """[N, H] -> [128, NH*KC*HW]: out[p, (hf, kc, j)] = Z[hf*HW+j, kc*128+p]."""
    return np.ascontiguousarray(
        Z.reshape(NH, HW, KC, 128).transpose(3, 0, 2, 1).reshape(128, NH * KC * HW))


def _make_l1_inputs(z_mp1, z_sc1, z_mp2, z_sc2, W1, b1, W2, b2):
    # zt layout [128, (t, kc, r)], t order [z_sc1, z_sc2, z_mp1, z_mp2]
    b2a = (b2 - W2.sum(axis=1)).astype(np.float32)
    w1t = _fm_pmajor(_f8(W1))            # W1.T[kc*128+p, o] = W1[o, ...]
    w2t = _fm_pmajor(W2.astype(NPBF16))
    b1c = np.ascontiguousarray(
        b1.reshape(KC, 128).T.astype(np.float32))    # [128, KC]
    b2r = np.ascontiguousarray(b2a.reshape(1, H).astype(NPBF16))
    onesr = np.ones((1, 128), NPBF16)
    zts = [_f8(z) for z in (z_sc1, z_sc2, z_mp1, z_mp2)]
    in1 = []
    for k in range(NC):
        sl = slice(k * RB, (k + 1) * RB)
        zt = np.concatenate([_fm_pmajor(z[sl]) for z in zts], axis=1)
        in1.append({"zt": np.ascontiguousarray(zt), "w1t": w1t, "w2t": w2t,
                    "b1": b1c, "b2r": b2r, "ones": onesr})
    return in1


def _unpack_pj(D):
    """[128, AT*H] sample-major dump -> [RB, H] matrix."""
    return D.reshape(128, AT, H).transpose(1, 0, 2).reshape(RB, H)


def _make_l2_inputs(res1, z_mp1, z_sc1, z_mp2, z_sc2):
    pj = [res1[k]["pj"] for k in range(NC)]
    Z1 = np.concatenate([_unpack_pj(pj[k][0]) for k in range(NC)], axis=0)
    Z2 = np.concatenate([_unpack_pj(pj[k][1]) for k in range(NC)], axis=0)
    r1f = _r_pmajor(Z1)
    r2f = _r_pmajor(Z2)

    A = _f8(z_mp1 + z_sc1)
    B = _f8(z_mp2 + z_sc2)
    btf = _r_pmajor(B)

    in2 = []
    for k in range(NC):
        sl = slice(k * RB, (k + 1) * RB)
        nrm = res1[k]["nrm"].astype(np.float32)       # [128, 2*AT] = |y|^2
        scl = np.ascontiguousarray(1.0 / (8.0 * TAU * np.sqrt(nrm) + 1e-30))
        in2.append({
            "lm1": _fm_pmajor(_unpack_pj(pj[k][2])),
            "lm2": _fm_pmajor(_unpack_pj(pj[k][3])),
            "scl": scl,
            "r1": r1f, "r2": r2f,
            "atb": _fm_pmajor(A[sl]),
            "btf": btf,
        })
    return in2


def _unpack_mc(D):
    """[2*AT, 128, HW] slot-major dump -> [RB, N] block (f32)."""
    return (D.reshape(AT, NH, 128, HW).transpose(0, 2, 1, 3)
             .reshape(RB, N).astype(np.float32))


def _finish(res2, pos1, pos2, pos_i, pos_j, neg_i, neg_j):
    losses = []
    for s, pos in ((0, pos1), (1, pos2)):
        key = "m1" if s == 0 else "m2"
        mf = np.concatenate([_unpack_mc(res2[k][key]) for k in range(NC)],
                            axis=0)
        num = np.einsum("ij,ij->i", mf, pos, dtype=np.float64)
        rsum = mf.sum(axis=1, dtype=np.float64)
        csum = mf.sum(axis=0, dtype=np.float64)
        numt = np.einsum("ij,ji->j", mf, pos, dtype=np.float64)
        lori_mp = -np.log(num / (rsum + 1e-8)).mean()
        lori_sc = -np.log(numt / (csum + 1e-8)).mean()
        losses.append(LAM * lori_mp + (1.0 - LAM) * lori_sc)

    C = np.concatenate([_unpack_mc(res2[k]["c"]) for k in range(NC)], axis=0)
    ip1 = C[pos_i, pos_j].astype(np.float64)
    ip2 = C[neg_i, neg_j].astype(np.float64)

    def logsig(x):
        return -np.logaddexp(0.0, -x)

    loss_main = -logsig(ip1).mean() + logsig(-ip2).mean()
    return np.float32(loss_main + losses[0] + losses[1])


def kernel(z_mp1, z_sc1, pos1, z_mp2, z_sc2, pos2,
           W1, b1, W2, b2, pos_i, pos_j, neg_i, neg_j):
    z_mp1 = np.asarray(z_mp1, np.float32)
    z_sc1 = np.asarray(z_sc1, np.float32)
    z_mp2 = np.asarray(z_mp2, np.float32)
    z_sc2 = np.asarray(z_sc2, np.float32)
    pos1 = np.asarray(pos1, np.float32)
    pos2 = np.asarray(pos2, np.float32)
    W1 = np.asarray(W1, np.float32)
    W2 = np.asarray(W2, np.float32)
    b1 = np.asarray(b1, np.float32)
    b2 = np.asarray(b2, np.float32)
    pos_i = np.asarray(pos_i)
    pos_j = np.asarray(pos_j)
    neg_i = np.asarray(neg_i)
    neg_j = np.asarray(neg_j)

    l1, l2 = _get_programs()
    cores = list(range(NC))

    in1 = _make_l1_inputs(z_mp1, z_sc1, z_mp2, z_sc2, W1, b1, W2, b2)
    res1 = run_bass_kernel_spmd(l1, in1, cores).results

    in2 = _make_l2_inputs(res1, z_mp1, z_sc1, z_mp2, z_sc2)
    res2 = run_bass_kernel_spmd(l2, in2, cores).results

    return _finish(res2, pos1, pos2, pos_i, pos_j, neg_i, neg_j)
